# revision 18
# baseline (speedup 1.0000x reference)
"""Chamfer distance (L1) Trainium2 Bass kernel — sorted-window version.

Problem: xyz1 (4, 8192, 3) fp32, xyz2 (4, 8192, 3) fp32 ->
scalar = mean_b[ mean_n min_m ||x1-x2|| + mean_m min_n ||x1-x2|| ].

Strategy:
 - 8 cores: core c handles batch b=c//2, N-half h=c%2 (4096 rows).
 - Host sorts both clouds by x. A 128-row tile of sorted rows only needs
   distances against a W-column rank-window around its aligned position:
   the device computes a banded (windowed) distance matrix instead of the
   full 4096x8192 block — W/8192 of the dense work.
 - Rigorous exactness: a windowed row/col min is provably exact whenever
   it is <= the x-gap to the nearest excluded column/row (|dx| lower-bounds
   the distance). The host flags the few points (~0.5%) violating this and
   recomputes them exactly in numpy. Result: exact up to fp16 rounding.
 - d2[n,m] = ||x1n||^2 + ||x2m||^2 - 2 x1n.x2m as ONE matmul with K=33 rows
   of 3-level split-precision bf16 (~fp32 accuracy at bf16 PE speed).
 - All minimums run as MAX over NEGATED distances (ACT drains PSUM with a
   free *-1): max-folds on DVE for both directions.
 - Per 2-tile pair: PE matmuls -> PSUM [128,2W]; ACT drains -d2 to fp16;
   DVE: one [128,2,W/2] max-halving into a wide rowhalf buffer (log-folded
   in deferred groups) + one contiguous max-fold into colacc per tile.
 - Col-min finish: PE transposes final colacc chunks (interleaved into the
   main loop), DVE free-axis max-reduce; GpSimd only memsets colacc.
 - sqrt + means + flag-fix on host over ~12K values per core.
"""

import sys

sys.path.insert(0, "/opt/trn_rl_repo")

import numpy as np
import ml_dtypes

import concourse.bass as bass
import concourse.bass_isa as bass_isa
import concourse.bacc as bacc
import concourse.mybir as mybir
import concourse.tile as tile
from concourse.bass_utils import run_bass_kernel_spmd

BF16 = mybir.dt.bfloat16
FP16 = mybir.dt.float16
FP32 = mybir.dt.float32
NP_BF16 = ml_dtypes.bfloat16

B, N, M = 4, 8192, 8192
N_CORES = 8
NC_N = N // 2  # 4096 rows per core
K_AUG = 33
TILES = NC_N // 128  # 32

W = 256  # window columns per 128-row tile (multiple of 128)
WC = W // 128  # window chunks
TPG = 2 if W >= 384 else 4  # tiles per psum/drain group
PW_ = W if 512 % W == 0 else -(-W // 512) * 512  # per-tile psum stride
LOC_M = (TILES - 1) * 128 + W  # local column space actually covered
TAIL_G = 4  # colacc chunks per tail transpose group
N_GROUPS = -(-(-(-LOC_M // 128)) // TAIL_G)
LOC_PAD = N_GROUPS * TAIL_G * 128  # padded local column space
PAD_X = 64.0  # x-coord of padding points (far away; d2 ~ 4e3)
INIT_F16 = -65504.0  # colacc init (negated-distance identity for max)
CHAIN_G = 8  # tiles per deferred row-min fold group
PW = PW_  # per-tile psum stride, PSUM-bank (512 fp32) safe


def build_program():
    nc = bacc.Bacc()

    lhs_d = nc.dram_tensor("lhs", [K_AUG, NC_N], BF16, kind="ExternalInput").ap()
    rhs_d = nc.dram_tensor("rhs", [K_AUG, LOC_PAD], BF16, kind="ExternalInput").ap()
    outm_d = nc.dram_tensor(
        "outm", [128, TILES + N_GROUPS * TAIL_G], FP16, kind="ExternalOutput"
    ).ap()

    amax = mybir.AluOpType.max
    ax_x = mybir.AxisListType.X
    HW = W // 2

    with tile.TileContext(nc) as tc:
        with (
            tc.tile_pool(name="const", bufs=1) as const_pool,
            tc.tile_pool(name="acc", bufs=1) as acc_pool,
            tc.tile_pool(name="drain", bufs=4) as drain_pool,
            tc.tile_pool(name="out", bufs=1) as out_pool,
            tc.tile_pool(name="mm", bufs=4, space="PSUM") as mm_pool,
        ):
            lhs_sb = const_pool.tile([K_AUG, NC_N], BF16)
            rhs_sb = const_pool.tile([K_AUG, LOC_PAD], BF16)
            # split input DMAs so the first tiles' slices land first; use
            # both hwdge queues (ACT frees earliest) so they overlap
            nc.scalar.dma_start(out=rhs_sb[:, :768], in_=rhs_d[:, :768])
            nc.sync.dma_start(out=lhs_sb[:, :512], in_=lhs_d[:, :512])
            nc.scalar.dma_start(out=rhs_sb[:, 768:1792], in_=rhs_d[:, 768:1792])
            nc.sync.dma_start(out=lhs_sb[:, 512:], in_=lhs_d[:, 512:])
            nc.scalar.dma_start(out=rhs_sb[:, 1792:], in_=rhs_d[:, 1792:])

            colacc = acc_pool.tile([128, LOC_PAD], FP16)
            rowhalf = acc_pool.tile([128, TILES * HW], FP16)
            trbuf = acc_pool.tile([128, LOC_PAD], FP16)
            outm_sb = out_pool.tile([128, TILES + N_GROUPS * TAIL_G], FP16)
            rowmin_sb = outm_sb[:, :TILES]
            colmin_sb = outm_sb[:, TILES:]

            # init col accumulator (GpSimd; first window's region first so
            # tile 0 can start folding early)
            nc.gpsimd.memset(colacc[:, :W], INIT_F16)
            nc.gpsimd.memset(colacc[:, W : 4 * W], INIT_F16)
            nc.gpsimd.memset(colacc[:, 4 * W :], INIT_F16)

            rhv = rowhalf.rearrange("p (t c) -> p t c", c=HW)
            rm = rowmin_sb.rearrange("p (t o) -> p t o", o=1)

            n_tail_emitted = 0
            n_chain_emitted = 0

            def emit_tail(g):
                # transpose each 128-chunk on the (idle) DMA xbar, then one
                # free-axis max-reduce on DVE
                for c4 in range(TAIL_G):
                    cc = g * TAIL_G + c4
                    nc.sync.dma_start_transpose(
                        out=trbuf[:, cc * 128 : (cc + 1) * 128],
                        in_=colacc[:, cc * 128 : (cc + 1) * 128],
                    )
                nc.vector.tensor_reduce(
                    colmin_sb[:, g * TAIL_G : (g + 1) * TAIL_G],
                    trbuf[:, g * TAIL_G * 128 : (g + 1) * TAIL_G * 128].rearrange(
                        "p (a b) -> p a b", b=128
                    ),
                    axis=ax_x,
                    op=amax,
                )

            for q in range(TILES // TPG):  # tile groups
                psum_t = mm_pool.tile([128, TPG * PW], FP32, tag="mm")
                for u in range(TPG):
                    t = TPG * q + u
                    lhs_i = lhs_sb[:, t * 128 : (t + 1) * 128]
                    splits = [512] * (W // 512) + ([W % 512] if W % 512 else [])
                    c0 = t * 128
                    o0 = u * PW  # bank-aligned: matmul must not straddle banks
                    for sz in splits:
                        nc.tensor.matmul(
                            psum_t[:, o0 : o0 + sz],
                            lhs_i,
                            rhs_sb[:, c0 : c0 + sz],
                        )
                        c0 += sz
                        o0 += sz
                drain = drain_pool.tile([128, TPG * PW], FP16)
                nc.scalar.mul(drain, psum_t, -1.0)  # drain = -d2

                # first row-direction halving for all group tiles in one op
                # (cols [W:PW) of each slot are pad garbage, never read)
                dvp = drain.rearrange("p (u c) -> p u c", u=TPG)
                nc.vector.tensor_tensor(
                    rhv[:, TPG * q : TPG * (q + 1), :],
                    dvp[:, :, :HW],
                    dvp[:, :, HW : 2 * HW],
                    amax,
                )

                # col-direction folds: one contiguous op per tile
                for u in range(TPG):
                    t = TPG * q + u
                    sl = colacc[:, t * 128 : t * 128 + W]
                    nc.vector.tensor_tensor(
                        sl, sl, drain[:, u * PW : u * PW + W], amax
                    )

                # transpose+reduce col groups once final (last touch: tile
                # of last chunk); 4-tile margin for cross-engine slack
                while (
                    n_tail_emitted < N_GROUPS
                    and min((n_tail_emitted + 1) * TAIL_G - 1, TILES - 1)
                    <= TPG * (q + 1) - 1 - 6
                ):
                    emit_tail(n_tail_emitted)
                    n_tail_emitted += 1

                # deferred row-direction fold chains per CHAIN_G tiles
                while (n_chain_emitted + 1) * CHAIN_G <= TPG * (q + 1):
                    j = n_chain_emitted
                    seg = rhv[:, j * CHAIN_G : (j + 1) * CHAIN_G, :]
                    k = HW // 2
                    while k % 2 == 0 and k > 48:
                        nc.vector.tensor_tensor(
                            seg[:, :, :k], seg[:, :, :k], seg[:, :, k : 2 * k], amax
                        )
                        k //= 2
                    nc.vector.tensor_reduce(
                        rm[:, j * CHAIN_G : (j + 1) * CHAIN_G, :],
                        seg[:, :, : 2 * k],
                        axis=ax_x,
                        op=amax,
                    )
                    n_chain_emitted += 1

            while n_tail_emitted < N_GROUPS:
                emit_tail(n_tail_emitted)
                n_tail_emitted += 1

            nc.sync.dma_start(out=outm_d, in_=outm_sb)

    nc.compile()
    return nc


def _split3(v):
    """v (f64 array) -> (hi, mid, lo) bf16 with hi+mid+lo ~= v (~26-bit)."""
    v = v.astype(np.float64)
    hi = v.astype(NP_BF16)
    r1 = v - hi.astype(np.float64)
    mid = r1.astype(NP_BF16)
    lo = (r1 - mid.astype(np.float64)).astype(NP_BF16)
    return hi, mid, lo


def _make_core_inputs(pts1, pts2):
    """pts1 (NC_N,3), pts2 (LOC_PAD,3) f64 -> lhs [33,NC_N], rhs [33,LOC_PAD] bf16.

    Row pairing (lhs_k paired with rhs_k), ordered so PE partial sums cancel
    early: d2 = sq1 + sq2 - 2*x1.x2 with 3-level splits.
    """
    a1 = _split3(pts1)
    a2 = _split3(pts2)
    n2 = [(-2.0 * p.astype(np.float64)).astype(NP_BF16) for p in a2]  # exact *-2
    sq1 = (pts1 * pts1).sum(-1)
    sq2 = (pts2 * pts2).sum(-1)
    s1 = _split3(sq1)
    s2 = _split3(sq2)

    ones_n = np.ones(pts1.shape[0], NP_BF16)
    ones_m = np.ones(pts2.shape[0], NP_BF16)

    lhs_rows = []
    rhs_rows = []

    def add(l, r):
        lhs_rows.append(l)
        rhs_rows.append(r)

    # big terms first, interleaved for cancellation
    add(s1[0], ones_m)
    for d in range(3):
        add(a1[0][:, d], n2[0][:, d])  # hi*hi
    add(ones_n, s2[0])
    # mid-level terms
    add(s1[1], ones_m)
    add(ones_n, s2[1])
    for d in range(3):
        add(a1[0][:, d], n2[1][:, d])  # hi*mid
    for d in range(3):
        add(a1[1][:, d], n2[0][:, d])  # mid*hi
    for d in range(3):
        add(a1[1][:, d], n2[1][:, d])  # mid*mid
    # low-level terms
    add(s1[2], ones_m)
    add(ones_n, s2[2])
    for d in range(3):
        add(a1[0][:, d], n2[2][:, d])  # hi*lo
    for d in range(3):
        add(a1[2][:, d], n2[0][:, d])  # lo*hi
    for d in range(3):
        add(a1[1][:, d], n2[2][:, d])  # mid*lo
    for d in range(3):
        add(a1[2][:, d], n2[1][:, d])  # lo*mid
    for d in range(3):
        add(a1[2][:, d], n2[2][:, d])  # lo*lo

    lhs = np.ascontiguousarray(np.stack(lhs_rows))
    rhs = np.ascontiguousarray(np.stack(rhs_rows))
    assert lhs.shape == (K_AUG, NC_N) and rhs.shape == (K_AUG, LOC_PAD)
    return lhs, rhs


def _exact_min_d2(a, b):
    """a (k,3), b (n,3) f64 -> (k,) min squared distance via gemm identity."""
    sa = (a * a).sum(-1)[:, None]
    sb = (b * b).sum(-1)[None, :]
    return (sa + sb - 2.0 * (a @ b.T)).min(1)


_CACHED_NC = None


def _get_nc():
    global _CACHED_NC
    if _CACHED_NC is None:
        _CACHED_NC = build_program()
    return _CACHED_NC


def _coverage_rows_for_cols(h, j_global):
    """For sorted col ranks j (array), rows covered by core-half h's windows.

    Returns (r_lo, r_hi) global sorted row ranks [r_lo, r_hi) covered; empty
    coverage gives r_lo >= r_hi.
    """
    loc = j_global + W // 2 - NC_N * h  # local column index
    t_lo = np.maximum((loc - W) // 128 + 1, 0)
    t_hi = np.minimum(loc // 128, TILES - 1)
    valid = (t_lo <= t_hi) & (loc >= 0) & (loc < LOC_M)
    r_lo = np.where(valid, NC_N * h + 128 * t_lo, 0)
    r_hi = np.where(valid, NC_N * h + 128 * t_hi + 128, 0)
    return r_lo, r_hi


def kernel(xyz1, xyz2, _return_timing=False, _trace=False):
    xyz1 = np.asarray(xyz1, dtype=np.float32)
    xyz2 = np.asarray(xyz2, dtype=np.float32)
    assert xyz1.shape == (B, N, 3) and xyz2.shape == (B, M, 3)

    xs1 = []
    xs2 = []
    in_maps = []
    for b in range(B):
        p = xyz1[b].astype(np.float64)
        g = xyz2[b].astype(np.float64)
        o1 = np.argsort(p[:, 0], kind="stable")
        o2 = np.argsort(g[:, 0], kind="stable")
        ps, gs = p[o1], g[o2]
        xs1.append(ps)
        xs2.append(gs)
        for h in range(2):
            rows = ps[h * NC_N : (h + 1) * NC_N]
            # local col l -> global sorted col l - W/2 + NC_N*h; pad outside
            l0 = -(W // 2) + NC_N * h
            cols = np.full((LOC_PAD, 3), 0.0, dtype=np.float64)
            cols[:, 0] = PAD_X
            gidx = np.arange(l0, l0 + LOC_PAD)
            sel = (gidx >= 0) & (gidx < M)
            cols[sel] = gs[gidx[sel]]
            lhs, rhs = _make_core_inputs(rows, cols)
            in_maps.append({"lhs": lhs, "rhs": rhs})

    nc = _get_nc()
    res = run_bass_kernel_spmd(
        nc, in_maps, core_ids=list(range(N_CORES)), trace=_trace
    )

    total = 0.0
    for b in range(B):
        ps, gs = xs1[b], xs2[b]
        x1, x2 = ps[:, 0], gs[:, 0]

        # ---- row mins (sorted order; device stores -d2) ----
        row_parts = []
        for h in range(2):
            r = res.results[2 * b + h]
            row_parts.append(
                -np.asarray(r["outm"])[:, :TILES].astype(np.float64).T.reshape(-1)
            )
        min1_d2 = np.concatenate(row_parts)  # (8192,) sorted rank order
        min1 = np.sqrt(np.maximum(min1_d2, 0.0))

        # ---- col mins ----
        col_d2 = np.full(M, np.inf)
        for h in range(2):
            r = res.results[2 * b + h]
            loc = (
                -np.asarray(r["outm"])[:, TILES:].astype(np.float64).T.reshape(-1)
            )
            l = np.arange(LOC_PAD)
            gidx = l - W // 2 + NC_N * h
            sel = (l < LOC_M) & (gidx >= 0) & (gidx < M)
            np.minimum.at(col_d2, gidx[sel], loc[sel])
        min2 = np.sqrt(np.maximum(col_d2, 0.0))

        # ---- flag + exact fix: rows ----
        r_rank = np.arange(N)
        t = (r_rank % NC_N) // 128
        h_arr = r_rank // NC_N
        glo = t * 128 + NC_N * h_arr - W // 2
        ghi = glo + W
        c_lo = np.maximum(glo, 0)
        c_hi = np.minimum(ghi, M)
        gapL = np.where(c_lo > 0, x1 - x2[np.maximum(c_lo - 1, 0)], np.inf)
        gapR = np.where(c_hi < M, x2[np.minimum(c_hi, M - 1)] - x1, np.inf)
        gap = np.maximum(np.minimum(gapL, gapR), 0.0)
        idx1 = np.where(min1 > gap * 0.999 - 1e-9)[0]
        if len(idx1):
            min1[idx1] = np.sqrt(np.maximum(_exact_min_d2(ps[idx1], gs), 0.0))

        # ---- flag + exact fix: cols ----
        j = np.arange(M)
        r0_lo, r0_hi = _coverage_rows_for_cols(0, j)
        r1_lo, r1_hi = _coverage_rows_for_cols(1, j)
        # union of [r0_lo,r0_hi) and [r1_lo,r1_hi); empty segments excluded
        e0 = r0_hi > r0_lo
        e1 = r1_hi > r1_lo
        lo_all = np.where(e0, r0_lo, r1_lo)
        hi_all = np.where(e1, r1_hi, r0_hi)
        gapLc = np.where(lo_all > 0, x2 - x1[np.maximum(lo_all - 1, 0)], np.inf)
        gapRc = np.where(hi_all < N, x1[np.minimum(hi_all, N - 1)] - x2, np.inf)
        # middle gap when both segments exist and don't abut
        mid_gap = np.full(M, np.inf)
        mid = e0 & e1 & (r0_hi < r1_lo)
        if mid.any():
            a = np.abs(x1[np.minimum(r0_hi, N - 1)] - x2)
            bb = np.abs(x1[np.maximum(r1_lo - 1, 0)] - x2)
            mid_gap = np.where(mid, np.minimum(a, bb), np.inf)
        gapc = np.maximum(np.minimum(np.minimum(gapLc, gapRc), mid_gap), 0.0)
        idx2 = np.where(min2 > gapc * 0.999 - 1e-9)[0]
        if len(idx2):
            min2[idx2] = np.sqrt(np.maximum(_exact_min_d2(gs[idx2], ps), 0.0))

        total += min1.mean() + min2.mean()

    out = np.asarray(total / B, dtype=np.float32)
    if _return_timing:
        return out, res
    return out


# revision 19
# speedup vs baseline: 1.6202x; 1.6202x over previous
"""Chamfer distance (L1) Trainium2 Bass kernel — sorted-window version.

Problem: xyz1 (4, 8192, 3) fp32, xyz2 (4, 8192, 3) fp32 ->
scalar = mean_b[ mean_n min_m ||x1-x2|| + mean_m min_n ||x1-x2|| ].

Strategy:
 - 8 cores: core c handles batch b=c//2, N-half h=c%2 (4096 rows).
 - Host sorts both clouds by x. A 128-row tile of sorted rows only needs
   distances against a W-column rank-window around its aligned position:
   the device computes a banded (windowed) distance matrix instead of the
   full 4096x8192 block — W/8192 of the dense work.
 - Rigorous exactness: a windowed row/col min is provably exact whenever
   it is <= the x-gap to the nearest excluded column/row (|dx| lower-bounds
   the distance). The host flags the few points (~0.5%) violating this and
   recomputes them exactly in numpy. Result: exact up to fp16 rounding.
 - d2[n,m] = ||x1n||^2 + ||x2m||^2 - 2 x1n.x2m as ONE matmul with K=33 rows
   of 3-level split-precision bf16 (~fp32 accuracy at bf16 PE speed).
 - All minimums run as MAX over NEGATED distances (ACT drains PSUM with a
   free *-1): max-folds on DVE for both directions.
 - Per 2-tile pair: PE matmuls -> PSUM [128,2W]; ACT drains -d2 to fp16;
   DVE: one [128,2,W/2] max-halving into a wide rowhalf buffer (log-folded
   in deferred groups) + one contiguous max-fold into colacc per tile.
 - Col-min finish: PE transposes final colacc chunks (interleaved into the
   main loop), DVE free-axis max-reduce; GpSimd only memsets colacc.
 - sqrt + means + flag-fix on host over ~12K values per core.
"""

import sys

sys.path.insert(0, "/opt/trn_rl_repo")

import numpy as np
import ml_dtypes

import concourse.bass as bass
import concourse.bass_isa as bass_isa
import concourse.bacc as bacc
import concourse.mybir as mybir
import concourse.tile as tile
from concourse.bass_utils import run_bass_kernel_spmd

BF16 = mybir.dt.bfloat16
FP16 = mybir.dt.float16
FP32 = mybir.dt.float32
NP_BF16 = ml_dtypes.bfloat16

B, N, M = 4, 8192, 8192
N_CORES = 8
NC_N = N // 2  # 4096 rows per core
K_AUG = 33
TILES = NC_N // 128  # 32

W = 256  # window columns per 128-row tile (multiple of 128)
WC = W // 128  # window chunks
TPG = 2 if W >= 384 else 4  # tiles per psum/drain group
PW_ = W if 512 % W == 0 else -(-W // 512) * 512  # per-tile psum stride
LOC_M = (TILES - 1) * 128 + W  # local column space actually covered
TAIL_G = 4  # colacc chunks per tail transpose group
N_GROUPS = -(-(-(-LOC_M // 128)) // TAIL_G)
LOC_PAD = N_GROUPS * TAIL_G * 128  # padded local column space
PAD_X = 64.0  # x-coord of padding points (far away; d2 ~ 4e3)
INIT_F16 = -65504.0  # colacc init (negated-distance identity for max)
CHAIN_G = 8  # tiles per deferred row-min fold group
PW = PW_  # per-tile psum stride, PSUM-bank (512 fp32) safe


def build_program():
    nc = bacc.Bacc()

    lhs_d = nc.dram_tensor("lhs", [K_AUG, NC_N], BF16, kind="ExternalInput").ap()
    rhs_d = nc.dram_tensor("rhs", [K_AUG, LOC_PAD], BF16, kind="ExternalInput").ap()
    ident_d = nc.dram_tensor("ident", [128, 128], FP16, kind="ExternalInput").ap()
    outm_d = nc.dram_tensor(
        "outm", [128, TILES + N_GROUPS * TAIL_G], FP16, kind="ExternalOutput"
    ).ap()

    amax = mybir.AluOpType.max
    ax_x = mybir.AxisListType.X
    HW = W // 2

    with tile.TileContext(nc) as tc:
        with (
            tc.tile_pool(name="const", bufs=1) as const_pool,
            tc.tile_pool(name="acc", bufs=1) as acc_pool,
            tc.tile_pool(name="drain", bufs=4) as drain_pool,
            tc.tile_pool(name="out", bufs=1) as out_pool,
            tc.tile_pool(name="mm", bufs=3, space="PSUM") as mm_pool,
            tc.tile_pool(name="tr", bufs=2, space="PSUM") as tr_pool,
        ):
            lhs_sb = const_pool.tile([K_AUG, NC_N], BF16)
            rhs_sb = const_pool.tile([K_AUG, LOC_PAD], BF16)
            ident_sb = const_pool.tile([128, 128], FP16)
            # split input DMAs so the first tiles' slices land first; use
            # both hwdge queues (ACT frees earliest) so they overlap
            nc.scalar.dma_start(out=rhs_sb[:, :768], in_=rhs_d[:, :768])
            nc.sync.dma_start(out=lhs_sb[:, :512], in_=lhs_d[:, :512])
            nc.scalar.dma_start(out=rhs_sb[:, 768:1792], in_=rhs_d[:, 768:1792])
            nc.sync.dma_start(out=lhs_sb[:, 512:], in_=lhs_d[:, 512:])
            nc.scalar.dma_start(out=rhs_sb[:, 1792:], in_=rhs_d[:, 1792:])
            nc.sync.dma_start(out=ident_sb, in_=ident_d)

            colacc = acc_pool.tile([128, LOC_PAD], FP16)
            rowhalf = acc_pool.tile([128, TILES * HW], FP16)
            outm_sb = out_pool.tile([128, TILES + N_GROUPS * TAIL_G], FP16)
            rowmin_sb = outm_sb[:, :TILES]
            colmin_sb = outm_sb[:, TILES:]

            # init col accumulator (GpSimd; first window's region first so
            # tile 0 can start folding early)
            nc.gpsimd.memset(colacc[:, :W], INIT_F16)
            nc.gpsimd.memset(colacc[:, W : 4 * W], INIT_F16)
            nc.gpsimd.memset(colacc[:, 4 * W :], INIT_F16)

            rhv = rowhalf.rearrange("p (t c) -> p t c", c=HW)
            rm = rowmin_sb.rearrange("p (t o) -> p t o", o=1)

            n_tail_emitted = 0
            n_chain_emitted = 0

            def emit_tail(g):
                tr_t = tr_pool.tile([128, TAIL_G * 128], FP16, tag="tr")
                for c4 in range(TAIL_G):
                    cc = g * TAIL_G + c4
                    nc.tensor.transpose(
                        tr_t[:, c4 * 128 : (c4 + 1) * 128],
                        colacc[:, cc * 128 : (cc + 1) * 128],
                        ident_sb,
                    )
                nc.vector.tensor_reduce(
                    colmin_sb[:, g * TAIL_G : (g + 1) * TAIL_G],
                    tr_t.rearrange("p (a b) -> p a b", b=128),
                    axis=ax_x,
                    op=amax,
                )

            for q in range(TILES // TPG):  # tile groups
                psum_t = mm_pool.tile([128, TPG * PW], FP32, tag="mm")
                for u in range(TPG):
                    t = TPG * q + u
                    lhs_i = lhs_sb[:, t * 128 : (t + 1) * 128]
                    splits = [512] * (W // 512) + ([W % 512] if W % 512 else [])
                    c0 = t * 128
                    o0 = u * PW  # bank-aligned: matmul must not straddle banks
                    for sz in splits:
                        nc.tensor.matmul(
                            psum_t[:, o0 : o0 + sz],
                            lhs_i,
                            rhs_sb[:, c0 : c0 + sz],
                        )
                        c0 += sz
                        o0 += sz
                drain = drain_pool.tile([128, TPG * PW], FP16)
                nc.scalar.mul(drain, psum_t, -1.0)  # drain = -d2

                # first row-direction halving for all group tiles in one op
                # (cols [W:PW) of each slot are pad garbage, never read)
                dvp = drain.rearrange("p (u c) -> p u c", u=TPG)
                nc.vector.tensor_tensor(
                    rhv[:, TPG * q : TPG * (q + 1), :],
                    dvp[:, :, :HW],
                    dvp[:, :, HW : 2 * HW],
                    amax,
                )

                # col-direction folds: one contiguous op per tile
                for u in range(TPG):
                    t = TPG * q + u
                    sl = colacc[:, t * 128 : t * 128 + W]
                    nc.vector.tensor_tensor(
                        sl, sl, drain[:, u * PW : u * PW + W], amax
                    )

                # transpose+reduce col groups once final (last touch: tile
                # of last chunk); 4-tile margin for cross-engine slack
                while (
                    n_tail_emitted < N_GROUPS
                    and min((n_tail_emitted + 1) * TAIL_G - 1, TILES - 1)
                    <= TPG * (q + 1) - 1 - 8
                ):
                    emit_tail(n_tail_emitted)
                    n_tail_emitted += 1

                # deferred row-direction fold chains per CHAIN_G tiles
                while (n_chain_emitted + 1) * CHAIN_G <= TPG * (q + 1):
                    j = n_chain_emitted
                    seg = rhv[:, j * CHAIN_G : (j + 1) * CHAIN_G, :]
                    k = HW // 2
                    while k % 2 == 0 and k > 48:
                        nc.vector.tensor_tensor(
                            seg[:, :, :k], seg[:, :, :k], seg[:, :, k : 2 * k], amax
                        )
                        k //= 2
                    nc.vector.tensor_reduce(
                        rm[:, j * CHAIN_G : (j + 1) * CHAIN_G, :],
                        seg[:, :, : 2 * k],
                        axis=ax_x,
                        op=amax,
                    )
                    n_chain_emitted += 1

            while n_tail_emitted < N_GROUPS:
                emit_tail(n_tail_emitted)
                n_tail_emitted += 1

            nc.sync.dma_start(out=outm_d, in_=outm_sb)

    nc.compile()
    return nc


def _split3(v):
    """v (f64 array) -> (hi, mid, lo) bf16 with hi+mid+lo ~= v (~26-bit)."""
    v = v.astype(np.float64)
    hi = v.astype(NP_BF16)
    r1 = v - hi.astype(np.float64)
    mid = r1.astype(NP_BF16)
    lo = (r1 - mid.astype(np.float64)).astype(NP_BF16)
    return hi, mid, lo


def _make_core_inputs(pts1, pts2):
    """pts1 (NC_N,3), pts2 (LOC_PAD,3) f64 -> lhs [33,NC_N], rhs [33,LOC_PAD] bf16.

    Row pairing (lhs_k paired with rhs_k), ordered so PE partial sums cancel
    early: d2 = sq1 + sq2 - 2*x1.x2 with 3-level splits.
    """
    a1 = _split3(pts1)
    a2 = _split3(pts2)
    n2 = [(-2.0 * p.astype(np.float64)).astype(NP_BF16) for p in a2]  # exact *-2
    sq1 = (pts1 * pts1).sum(-1)
    sq2 = (pts2 * pts2).sum(-1)
    s1 = _split3(sq1)
    s2 = _split3(sq2)

    ones_n = np.ones(pts1.shape[0], NP_BF16)
    ones_m = np.ones(pts2.shape[0], NP_BF16)

    lhs_rows = []
    rhs_rows = []

    def add(l, r):
        lhs_rows.append(l)
        rhs_rows.append(r)

    # big terms first, interleaved for cancellation
    add(s1[0], ones_m)
    for d in range(3):
        add(a1[0][:, d], n2[0][:, d])  # hi*hi
    add(ones_n, s2[0])
    # mid-level terms
    add(s1[1], ones_m)
    add(ones_n, s2[1])
    for d in range(3):
        add(a1[0][:, d], n2[1][:, d])  # hi*mid
    for d in range(3):
        add(a1[1][:, d], n2[0][:, d])  # mid*hi
    for d in range(3):
        add(a1[1][:, d], n2[1][:, d])  # mid*mid
    # low-level terms
    add(s1[2], ones_m)
    add(ones_n, s2[2])
    for d in range(3):
        add(a1[0][:, d], n2[2][:, d])  # hi*lo
    for d in range(3):
        add(a1[2][:, d], n2[0][:, d])  # lo*hi
    for d in range(3):
        add(a1[1][:, d], n2[2][:, d])  # mid*lo
    for d in range(3):
        add(a1[2][:, d], n2[1][:, d])  # lo*mid
    for d in range(3):
        add(a1[2][:, d], n2[2][:, d])  # lo*lo

    lhs = np.ascontiguousarray(np.stack(lhs_rows))
    rhs = np.ascontiguousarray(np.stack(rhs_rows))
    assert lhs.shape == (K_AUG, NC_N) and rhs.shape == (K_AUG, LOC_PAD)
    return lhs, rhs


def _exact_min_d2(a, b):
    """a (k,3), b (n,3) f64 -> (k,) min squared distance via gemm identity."""
    sa = (a * a).sum(-1)[:, None]
    sb = (b * b).sum(-1)[None, :]
    return (sa + sb - 2.0 * (a @ b.T)).min(1)


_CACHED_NC = None


def _get_nc():
    global _CACHED_NC
    if _CACHED_NC is None:
        _CACHED_NC = build_program()
    return _CACHED_NC


def _coverage_rows_for_cols(h, j_global):
    """For sorted col ranks j (array), rows covered by core-half h's windows.

    Returns (r_lo, r_hi) global sorted row ranks [r_lo, r_hi) covered; empty
    coverage gives r_lo >= r_hi.
    """
    loc = j_global + W // 2 - NC_N * h  # local column index
    t_lo = np.maximum((loc - W) // 128 + 1, 0)
    t_hi = np.minimum(loc // 128, TILES - 1)
    valid = (t_lo <= t_hi) & (loc >= 0) & (loc < LOC_M)
    r_lo = np.where(valid, NC_N * h + 128 * t_lo, 0)
    r_hi = np.where(valid, NC_N * h + 128 * t_hi + 128, 0)
    return r_lo, r_hi


def kernel(xyz1, xyz2, _return_timing=False, _trace=False):
    xyz1 = np.asarray(xyz1, dtype=np.float32)
    xyz2 = np.asarray(xyz2, dtype=np.float32)
    assert xyz1.shape == (B, N, 3) and xyz2.shape == (B, M, 3)

    ident = np.eye(128, dtype=np.float16)
    xs1 = []
    xs2 = []
    in_maps = []
    for b in range(B):
        p = xyz1[b].astype(np.float64)
        g = xyz2[b].astype(np.float64)
        o1 = np.argsort(p[:, 0], kind="stable")
        o2 = np.argsort(g[:, 0], kind="stable")
        ps, gs = p[o1], g[o2]
        xs1.append(ps)
        xs2.append(gs)
        for h in range(2):
            rows = ps[h * NC_N : (h + 1) * NC_N]
            # local col l -> global sorted col l - W/2 + NC_N*h; pad outside
            l0 = -(W // 2) + NC_N * h
            cols = np.full((LOC_PAD, 3), 0.0, dtype=np.float64)
            cols[:, 0] = PAD_X
            gidx = np.arange(l0, l0 + LOC_PAD)
            sel = (gidx >= 0) & (gidx < M)
            cols[sel] = gs[gidx[sel]]
            lhs, rhs = _make_core_inputs(rows, cols)
            in_maps.append({"lhs": lhs, "rhs": rhs, "ident": ident})

    nc = _get_nc()
    res = run_bass_kernel_spmd(
        nc, in_maps, core_ids=list(range(N_CORES)), trace=_trace
    )

    total = 0.0
    for b in range(B):
        ps, gs = xs1[b], xs2[b]
        x1, x2 = ps[:, 0], gs[:, 0]

        # ---- row mins (sorted order; device stores -d2) ----
        row_parts = []
        for h in range(2):
            r = res.results[2 * b + h]
            row_parts.append(
                -np.asarray(r["outm"])[:, :TILES].astype(np.float64).T.reshape(-1)
            )
        min1_d2 = np.concatenate(row_parts)  # (8192,) sorted rank order
        min1 = np.sqrt(np.maximum(min1_d2, 0.0))

        # ---- col mins ----
        col_d2 = np.full(M, np.inf)
        for h in range(2):
            r = res.results[2 * b + h]
            loc = (
                -np.asarray(r["outm"])[:, TILES:].astype(np.float64).T.reshape(-1)
            )
            l = np.arange(LOC_PAD)
            gidx = l - W // 2 + NC_N * h
            sel = (l < LOC_M) & (gidx >= 0) & (gidx < M)
            np.minimum.at(col_d2, gidx[sel], loc[sel])
        min2 = np.sqrt(np.maximum(col_d2, 0.0))

        # ---- flag + exact fix: rows ----
        r_rank = np.arange(N)
        t = (r_rank % NC_N) // 128
        h_arr = r_rank // NC_N
        glo = t * 128 + NC_N * h_arr - W // 2
        ghi = glo + W
        c_lo = np.maximum(glo, 0)
        c_hi = np.minimum(ghi, M)
        gapL = np.where(c_lo > 0, x1 - x2[np.maximum(c_lo - 1, 0)], np.inf)
        gapR = np.where(c_hi < M, x2[np.minimum(c_hi, M - 1)] - x1, np.inf)
        gap = np.maximum(np.minimum(gapL, gapR), 0.0)
        idx1 = np.where(min1 > gap * 0.999 - 1e-9)[0]
        if len(idx1):
            min1[idx1] = np.sqrt(np.maximum(_exact_min_d2(ps[idx1], gs), 0.0))

        # ---- flag + exact fix: cols ----
        j = np.arange(M)
        r0_lo, r0_hi = _coverage_rows_for_cols(0, j)
        r1_lo, r1_hi = _coverage_rows_for_cols(1, j)
        # union of [r0_lo,r0_hi) and [r1_lo,r1_hi); empty segments excluded
        e0 = r0_hi > r0_lo
        e1 = r1_hi > r1_lo
        lo_all = np.where(e0, r0_lo, r1_lo)
        hi_all = np.where(e1, r1_hi, r0_hi)
        gapLc = np.where(lo_all > 0, x2 - x1[np.maximum(lo_all - 1, 0)], np.inf)
        gapRc = np.where(hi_all < N, x1[np.minimum(hi_all, N - 1)] - x2, np.inf)
        # middle gap when both segments exist and don't abut
        mid_gap = np.full(M, np.inf)
        mid = e0 & e1 & (r0_hi < r1_lo)
        if mid.any():
            a = np.abs(x1[np.minimum(r0_hi, N - 1)] - x2)
            bb = np.abs(x1[np.maximum(r1_lo - 1, 0)] - x2)
            mid_gap = np.where(mid, np.minimum(a, bb), np.inf)
        gapc = np.maximum(np.minimum(np.minimum(gapLc, gapRc), mid_gap), 0.0)
        idx2 = np.where(min2 > gapc * 0.999 - 1e-9)[0]
        if len(idx2):
            min2[idx2] = np.sqrt(np.maximum(_exact_min_d2(gs[idx2], ps), 0.0))

        total += min1.mean() + min2.mean()

    out = np.asarray(total / B, dtype=np.float32)
    if _return_timing:
        return out, res
    return out


# revision 20
# speedup vs baseline: 1.6210x; 1.0005x over previous
"""Chamfer distance (L1) Trainium2 Bass kernel — sorted-window version.

Problem: xyz1 (4, 8192, 3) fp32, xyz2 (4, 8192, 3) fp32 ->
scalar = mean_b[ mean_n min_m ||x1-x2|| + mean_m min_n ||x1-x2|| ].

Strategy:
 - 8 cores: core c handles batch b=c//2, N-half h=c%2 (4096 rows).
 - Host sorts both clouds by x. A 128-row tile of sorted rows only needs
   distances against a W-column rank-window around its aligned position:
   the device computes a banded (windowed) distance matrix instead of the
   full 4096x8192 block — W/8192 of the dense work.
 - Rigorous exactness: a windowed row/col min is provably exact whenever
   it is <= the x-gap to the nearest excluded column/row (|dx| lower-bounds
   the distance). The host flags the few points (~0.5%) violating this and
   recomputes them exactly in numpy. Result: exact up to fp16 rounding.
 - d2[n,m] = ||x1n||^2 + ||x2m||^2 - 2 x1n.x2m as ONE matmul with K=33 rows
   of 3-level split-precision bf16 (~fp32 accuracy at bf16 PE speed).
 - All minimums run as MAX over NEGATED distances (ACT drains PSUM with a
   free *-1): max-folds on DVE for both directions.
 - Per 2-tile pair: PE matmuls -> PSUM [128,2W]; ACT drains -d2 to fp16;
   DVE: one [128,2,W/2] max-halving into a wide rowhalf buffer (log-folded
   in deferred groups) + one contiguous max-fold into colacc per tile.
 - Col-min finish: PE transposes final colacc chunks (interleaved into the
   main loop), DVE free-axis max-reduce; GpSimd only memsets colacc.
 - sqrt + means + flag-fix on host over ~12K values per core.
"""

import sys

sys.path.insert(0, "/opt/trn_rl_repo")

import numpy as np
import ml_dtypes

import concourse.bass as bass
import concourse.bass_isa as bass_isa
import concourse.bacc as bacc
import concourse.mybir as mybir
import concourse.tile as tile
from concourse.bass_utils import run_bass_kernel_spmd

BF16 = mybir.dt.bfloat16
FP16 = mybir.dt.float16
FP32 = mybir.dt.float32
NP_BF16 = ml_dtypes.bfloat16

B, N, M = 4, 8192, 8192
N_CORES = 8
NC_N = N // 2  # 4096 rows per core
K_AUG = 33
TILES = NC_N // 128  # 32

W = 256  # window columns per 128-row tile (multiple of 128)
WC = W // 128  # window chunks
TPG = 2 if W >= 384 else 4  # tiles per psum/drain group
PW_ = W if 512 % W == 0 else -(-W // 512) * 512  # per-tile psum stride
LOC_M = (TILES - 1) * 128 + W  # local column space actually covered
TAIL_G = 4  # colacc chunks per tail transpose group
N_GROUPS = -(-(-(-LOC_M // 128)) // TAIL_G)
LOC_PAD = N_GROUPS * TAIL_G * 128  # padded local column space
PAD_X = 64.0  # x-coord of padding points (far away; d2 ~ 4e3)
INIT_F16 = -65504.0  # colacc init (negated-distance identity for max)
CHAIN_G = 8  # tiles per deferred row-min fold group
N_GP = 5  # leading col-min groups reduced on GpSimd (idle mid-loop)
PW = PW_  # per-tile psum stride, PSUM-bank (512 fp32) safe


def build_program():
    nc = bacc.Bacc()

    lhs_d = nc.dram_tensor("lhs", [K_AUG, NC_N], BF16, kind="ExternalInput").ap()
    rhs_d = nc.dram_tensor("rhs", [K_AUG, LOC_PAD], BF16, kind="ExternalInput").ap()
    ident_d = nc.dram_tensor("ident", [128, 128], FP16, kind="ExternalInput").ap()
    outm_d = nc.dram_tensor(
        "outm", [128, TILES + N_GROUPS * TAIL_G], FP16, kind="ExternalOutput"
    ).ap()
    colgp_d = nc.dram_tensor(
        "colgp", [1, N_GP * TAIL_G * 128], FP16, kind="ExternalOutput"
    ).ap()

    amax = mybir.AluOpType.max
    ax_x = mybir.AxisListType.X
    HW = W // 2

    with tile.TileContext(nc) as tc:
        with (
            tc.tile_pool(name="const", bufs=1) as const_pool,
            tc.tile_pool(name="acc", bufs=1) as acc_pool,
            tc.tile_pool(name="drain", bufs=4) as drain_pool,
            tc.tile_pool(name="out", bufs=1) as out_pool,
            tc.tile_pool(name="mm", bufs=3, space="PSUM") as mm_pool,
            tc.tile_pool(name="tr", bufs=2, space="PSUM") as tr_pool,
        ):
            lhs_sb = const_pool.tile([K_AUG, NC_N], BF16)
            rhs_sb = const_pool.tile([K_AUG, LOC_PAD], BF16)
            ident_sb = const_pool.tile([128, 128], FP16)
            # split input DMAs so the first tiles' slices land first; use
            # both hwdge queues (ACT frees earliest) so they overlap
            nc.scalar.dma_start(out=rhs_sb[:, :640], in_=rhs_d[:, :640])
            nc.sync.dma_start(out=lhs_sb[:, :512], in_=lhs_d[:, :512])
            nc.scalar.dma_start(out=rhs_sb[:, 640:1664], in_=rhs_d[:, 640:1664])
            nc.sync.dma_start(out=lhs_sb[:, 512:], in_=lhs_d[:, 512:])
            nc.scalar.dma_start(out=rhs_sb[:, 1664:], in_=rhs_d[:, 1664:])
            nc.sync.dma_start(out=ident_sb, in_=ident_d)

            colacc = acc_pool.tile([128, LOC_PAD], FP16)
            rowhalf = acc_pool.tile([128, TILES * HW], FP16)
            colred = acc_pool.tile([128, N_GP * TAIL_G * 128], FP16)
            outm_sb = out_pool.tile([128, TILES + N_GROUPS * TAIL_G], FP16)
            rowmin_sb = outm_sb[:, :TILES]
            colmin_sb = outm_sb[:, TILES:]

            # init col accumulator (GpSimd; first window's region first so
            # tile 0 can start folding early)
            nc.gpsimd.memset(colacc[:, :W], INIT_F16)
            nc.gpsimd.memset(colacc[:, W : 4 * W], INIT_F16)
            nc.gpsimd.memset(colacc[:, 4 * W :], INIT_F16)

            rhv = rowhalf.rearrange("p (t c) -> p t c", c=HW)
            rm = rowmin_sb.rearrange("p (t o) -> p t o", o=1)

            n_tail_emitted = 0
            n_chain_emitted = 0

            def emit_tail(g):
                if g < N_GP:
                    # partition-direction max on the otherwise idle GpSimd
                    s0 = g * TAIL_G * 128
                    s1 = (g + 1) * TAIL_G * 128
                    nc.gpsimd.partition_all_reduce(
                        colred[:, s0:s1],
                        colacc[:, s0:s1],
                        128,
                        bass_isa.ReduceOp.max,
                    )
                    return
                tr_t = tr_pool.tile([128, TAIL_G * 128], FP16, tag="tr")
                for c4 in range(TAIL_G):
                    cc = g * TAIL_G + c4
                    nc.tensor.transpose(
                        tr_t[:, c4 * 128 : (c4 + 1) * 128],
                        colacc[:, cc * 128 : (cc + 1) * 128],
                        ident_sb,
                    )
                nc.vector.tensor_reduce(
                    colmin_sb[:, g * TAIL_G : (g + 1) * TAIL_G],
                    tr_t.rearrange("p (a b) -> p a b", b=128),
                    axis=ax_x,
                    op=amax,
                )

            for q in range(TILES // TPG):  # tile groups
                psum_t = mm_pool.tile([128, TPG * PW], FP32, tag="mm")
                for u in range(TPG):
                    t = TPG * q + u
                    lhs_i = lhs_sb[:, t * 128 : (t + 1) * 128]
                    splits = [512] * (W // 512) + ([W % 512] if W % 512 else [])
                    c0 = t * 128
                    o0 = u * PW  # bank-aligned: matmul must not straddle banks
                    for sz in splits:
                        nc.tensor.matmul(
                            psum_t[:, o0 : o0 + sz],
                            lhs_i,
                            rhs_sb[:, c0 : c0 + sz],
                        )
                        c0 += sz
                        o0 += sz
                drain = drain_pool.tile([128, TPG * PW], FP16)
                nc.scalar.mul(drain, psum_t, -1.0)  # drain = -d2

                # first row-direction halving for all group tiles in one op
                # (cols [W:PW) of each slot are pad garbage, never read)
                dvp = drain.rearrange("p (u c) -> p u c", u=TPG)
                nc.vector.tensor_tensor(
                    rhv[:, TPG * q : TPG * (q + 1), :],
                    dvp[:, :, :HW],
                    dvp[:, :, HW : 2 * HW],
                    amax,
                )

                # col-direction folds: one contiguous op per tile
                for u in range(TPG):
                    t = TPG * q + u
                    sl = colacc[:, t * 128 : t * 128 + W]
                    nc.vector.tensor_tensor(
                        sl, sl, drain[:, u * PW : u * PW + W], amax
                    )

                # transpose+reduce col groups once final (last touch: tile
                # of last chunk); 4-tile margin for cross-engine slack
                while (
                    n_tail_emitted < N_GP
                    and min((n_tail_emitted + 1) * TAIL_G - 1, TILES - 1)
                    <= TPG * (q + 1) - 1 - 6
                ):
                    emit_tail(n_tail_emitted)
                    n_tail_emitted += 1

                # deferred row-direction fold chains per CHAIN_G tiles
                while (n_chain_emitted + 1) * CHAIN_G <= TPG * (q + 1):
                    j = n_chain_emitted
                    nc.vector.tensor_reduce(
                        rm[:, j * CHAIN_G : (j + 1) * CHAIN_G, :],
                        rhv[:, j * CHAIN_G : (j + 1) * CHAIN_G, :],
                        axis=ax_x,
                        op=amax,
                    )
                    n_chain_emitted += 1

            while n_tail_emitted < N_GROUPS:
                emit_tail(n_tail_emitted)
                n_tail_emitted += 1

            nc.sync.dma_start(out=colgp_d, in_=colred[0:1, :])
            nc.sync.dma_start(out=outm_d, in_=outm_sb)

    nc.compile()
    return nc


def _split3(v):
    """v (f64 array) -> (hi, mid, lo) bf16 with hi+mid+lo ~= v (~26-bit)."""
    v = v.astype(np.float64)
    hi = v.astype(NP_BF16)
    r1 = v - hi.astype(np.float64)
    mid = r1.astype(NP_BF16)
    lo = (r1 - mid.astype(np.float64)).astype(NP_BF16)
    return hi, mid, lo


def _make_core_inputs(pts1, pts2):
    """pts1 (NC_N,3), pts2 (LOC_PAD,3) f64 -> lhs [33,NC_N], rhs [33,LOC_PAD] bf16.

    Row pairing (lhs_k paired with rhs_k), ordered so PE partial sums cancel
    early: d2 = sq1 + sq2 - 2*x1.x2 with 3-level splits.
    """
    a1 = _split3(pts1)
    a2 = _split3(pts2)
    n2 = [(-2.0 * p.astype(np.float64)).astype(NP_BF16) for p in a2]  # exact *-2
    sq1 = (pts1 * pts1).sum(-1)
    sq2 = (pts2 * pts2).sum(-1)
    s1 = _split3(sq1)
    s2 = _split3(sq2)

    ones_n = np.ones(pts1.shape[0], NP_BF16)
    ones_m = np.ones(pts2.shape[0], NP_BF16)

    lhs_rows = []
    rhs_rows = []

    def add(l, r):
        lhs_rows.append(l)
        rhs_rows.append(r)

    # big terms first, interleaved for cancellation
    add(s1[0], ones_m)
    for d in range(3):
        add(a1[0][:, d], n2[0][:, d])  # hi*hi
    add(ones_n, s2[0])
    # mid-level terms
    add(s1[1], ones_m)
    add(ones_n, s2[1])
    for d in range(3):
        add(a1[0][:, d], n2[1][:, d])  # hi*mid
    for d in range(3):
        add(a1[1][:, d], n2[0][:, d])  # mid*hi
    for d in range(3):
        add(a1[1][:, d], n2[1][:, d])  # mid*mid
    # low-level terms
    add(s1[2], ones_m)
    add(ones_n, s2[2])
    for d in range(3):
        add(a1[0][:, d], n2[2][:, d])  # hi*lo
    for d in range(3):
        add(a1[2][:, d], n2[0][:, d])  # lo*hi
    for d in range(3):
        add(a1[1][:, d], n2[2][:, d])  # mid*lo
    for d in range(3):
        add(a1[2][:, d], n2[1][:, d])  # lo*mid
    for d in range(3):
        add(a1[2][:, d], n2[2][:, d])  # lo*lo

    lhs = np.ascontiguousarray(np.stack(lhs_rows))
    rhs = np.ascontiguousarray(np.stack(rhs_rows))
    assert lhs.shape == (K_AUG, NC_N) and rhs.shape == (K_AUG, LOC_PAD)
    return lhs, rhs


def _exact_min_d2(a, b):
    """a (k,3), b (n,3) f64 -> (k,) min squared distance via gemm identity."""
    sa = (a * a).sum(-1)[:, None]
    sb = (b * b).sum(-1)[None, :]
    return (sa + sb - 2.0 * (a @ b.T)).min(1)


_CACHED_NC = None


def _get_nc():
    global _CACHED_NC
    if _CACHED_NC is None:
        _CACHED_NC = build_program()
    return _CACHED_NC


def _coverage_rows_for_cols(h, j_global):
    """For sorted col ranks j (array), rows covered by core-half h's windows.

    Returns (r_lo, r_hi) global sorted row ranks [r_lo, r_hi) covered; empty
    coverage gives r_lo >= r_hi.
    """
    loc = j_global + W // 2 - NC_N * h  # local column index
    t_lo = np.maximum((loc - W) // 128 + 1, 0)
    t_hi = np.minimum(loc // 128, TILES - 1)
    valid = (t_lo <= t_hi) & (loc >= 0) & (loc < LOC_M)
    r_lo = np.where(valid, NC_N * h + 128 * t_lo, 0)
    r_hi = np.where(valid, NC_N * h + 128 * t_hi + 128, 0)
    return r_lo, r_hi


def kernel(xyz1, xyz2, _return_timing=False, _trace=False):
    xyz1 = np.asarray(xyz1, dtype=np.float32)
    xyz2 = np.asarray(xyz2, dtype=np.float32)
    assert xyz1.shape == (B, N, 3) and xyz2.shape == (B, M, 3)

    ident = np.eye(128, dtype=np.float16)
    xs1 = []
    xs2 = []
    in_maps = []
    for b in range(B):
        p = xyz1[b].astype(np.float64)
        g = xyz2[b].astype(np.float64)
        o1 = np.argsort(p[:, 0], kind="stable")
        o2 = np.argsort(g[:, 0], kind="stable")
        ps, gs = p[o1], g[o2]
        xs1.append(ps)
        xs2.append(gs)
        for h in range(2):
            rows = ps[h * NC_N : (h + 1) * NC_N]
            # local col l -> global sorted col l - W/2 + NC_N*h; pad outside
            l0 = -(W // 2) + NC_N * h
            cols = np.full((LOC_PAD, 3), 0.0, dtype=np.float64)
            cols[:, 0] = PAD_X
            gidx = np.arange(l0, l0 + LOC_PAD)
            sel = (gidx >= 0) & (gidx < M)
            cols[sel] = gs[gidx[sel]]
            lhs, rhs = _make_core_inputs(rows, cols)
            in_maps.append({"lhs": lhs, "rhs": rhs, "ident": ident})

    nc = _get_nc()
    res = run_bass_kernel_spmd(
        nc, in_maps, core_ids=list(range(N_CORES)), trace=_trace
    )

    total = 0.0
    for b in range(B):
        ps, gs = xs1[b], xs2[b]
        x1, x2 = ps[:, 0], gs[:, 0]

        # ---- row mins (sorted order; device stores -d2) ----
        row_parts = []
        for h in range(2):
            r = res.results[2 * b + h]
            row_parts.append(
                -np.asarray(r["outm"])[:, :TILES].astype(np.float64).T.reshape(-1)
            )
        min1_d2 = np.concatenate(row_parts)  # (8192,) sorted rank order
        min1 = np.sqrt(np.maximum(min1_d2, 0.0))

        # ---- col mins ----
        col_d2 = np.full(M, np.inf)
        for h in range(2):
            r = res.results[2 * b + h]
            gp = -np.asarray(r["colgp"]).astype(np.float64).reshape(-1)
            pe = (
                -np.asarray(r["outm"])[:, TILES:].astype(np.float64).T.reshape(-1)
            )
            loc = np.concatenate([gp, pe[N_GP * TAIL_G * 128 :]])
            l = np.arange(LOC_PAD)
            gidx = l - W // 2 + NC_N * h
            sel = (l < LOC_M) & (gidx >= 0) & (gidx < M)
            np.minimum.at(col_d2, gidx[sel], loc[sel])
        min2 = np.sqrt(np.maximum(col_d2, 0.0))

        # ---- flag + exact fix: rows ----
        r_rank = np.arange(N)
        t = (r_rank % NC_N) // 128
        h_arr = r_rank // NC_N
        glo = t * 128 + NC_N * h_arr - W // 2
        ghi = glo + W
        c_lo = np.maximum(glo, 0)
        c_hi = np.minimum(ghi, M)
        gapL = np.where(c_lo > 0, x1 - x2[np.maximum(c_lo - 1, 0)], np.inf)
        gapR = np.where(c_hi < M, x2[np.minimum(c_hi, M - 1)] - x1, np.inf)
        gap = np.maximum(np.minimum(gapL, gapR), 0.0)
        idx1 = np.where(min1 > gap * 0.999 - 1e-9)[0]
        if len(idx1):
            min1[idx1] = np.sqrt(np.maximum(_exact_min_d2(ps[idx1], gs), 0.0))

        # ---- flag + exact fix: cols ----
        j = np.arange(M)
        r0_lo, r0_hi = _coverage_rows_for_cols(0, j)
        r1_lo, r1_hi = _coverage_rows_for_cols(1, j)
        # union of [r0_lo,r0_hi) and [r1_lo,r1_hi); empty segments excluded
        e0 = r0_hi > r0_lo
        e1 = r1_hi > r1_lo
        lo_all = np.where(e0, r0_lo, r1_lo)
        hi_all = np.where(e1, r1_hi, r0_hi)
        gapLc = np.where(lo_all > 0, x2 - x1[np.maximum(lo_all - 1, 0)], np.inf)
        gapRc = np.where(hi_all < N, x1[np.minimum(hi_all, N - 1)] - x2, np.inf)
        # middle gap when both segments exist and don't abut
        mid_gap = np.full(M, np.inf)
        mid = e0 & e1 & (r0_hi < r1_lo)
        if mid.any():
            a = np.abs(x1[np.minimum(r0_hi, N - 1)] - x2)
            bb = np.abs(x1[np.maximum(r1_lo - 1, 0)] - x2)
            mid_gap = np.where(mid, np.minimum(a, bb), np.inf)
        gapc = np.maximum(np.minimum(np.minimum(gapLc, gapRc), mid_gap), 0.0)
        idx2 = np.where(min2 > gapc * 0.999 - 1e-9)[0]
        if len(idx2):
            min2[idx2] = np.sqrt(np.maximum(_exact_min_d2(gs[idx2], ps), 0.0))

        total += min1.mean() + min2.mean()

    out = np.asarray(total / B, dtype=np.float32)
    if _return_timing:
        return out, res
    return out


# revision 22
# speedup vs baseline: 1.7915x; 1.1052x over previous
"""Chamfer distance (L1) Trainium2 Bass kernel — sorted-window version.

Problem: xyz1 (4, 8192, 3) fp32, xyz2 (4, 8192, 3) fp32 ->
scalar = mean_b[ mean_n min_m ||x1-x2|| + mean_m min_n ||x1-x2|| ].

Strategy:
 - 8 cores: core c handles batch b=c//2, N-half h=c%2 (4096 rows).
 - Host sorts both clouds by x. A 128-row tile of sorted rows only needs
   distances against a W-column rank-window around its aligned position:
   the device computes a banded (windowed) distance matrix instead of the
   full 4096x8192 block — W/8192 of the dense work.
 - Rigorous exactness: a windowed row/col min is provably exact whenever
   it is <= the x-gap to the nearest excluded column/row (|dx| lower-bounds
   the distance). The host flags the few points (~0.5%) violating this and
   recomputes them exactly in numpy. Result: exact up to fp16 rounding.
 - d2[n,m] = ||x1n||^2 + ||x2m||^2 - 2 x1n.x2m as ONE matmul with K=33 rows
   of 3-level split-precision bf16 (~fp32 accuracy at bf16 PE speed).
 - All minimums run as MAX over NEGATED distances (ACT drains PSUM with a
   free *-1): max-folds on DVE for both directions.
 - Per 2-tile pair: PE matmuls -> PSUM [128,2W]; ACT drains -d2 to fp16;
   DVE: one [128,2,W/2] max-halving into a wide rowhalf buffer (log-folded
   in deferred groups) + one contiguous max-fold into colacc per tile.
 - Col-min finish: PE transposes final colacc chunks (interleaved into the
   main loop), DVE free-axis max-reduce; GpSimd only memsets colacc.
 - sqrt + means + flag-fix on host over ~12K values per core.
"""

import sys

sys.path.insert(0, "/opt/trn_rl_repo")

import numpy as np
import ml_dtypes

import concourse.bass as bass
import concourse.bass_isa as bass_isa
import concourse.bacc as bacc
import concourse.mybir as mybir
import concourse.tile as tile
from concourse.bass_utils import run_bass_kernel_spmd

BF16 = mybir.dt.bfloat16
FP16 = mybir.dt.float16
FP32 = mybir.dt.float32
NP_BF16 = ml_dtypes.bfloat16

B, N, M = 4, 8192, 8192
N_CORES = 8
NC_N = N // 2  # 4096 rows per core
K_AUG = 33
TILES = NC_N // 128  # 32

W = 256  # window columns per 128-row tile (multiple of 128)
WC = W // 128  # window chunks
TPG = 2 if W >= 384 else 4  # tiles per psum/drain group
PW_ = W if 512 % W == 0 else -(-W // 512) * 512  # per-tile psum stride
LOC_M = (TILES - 1) * 128 + W  # local column space actually covered
TAIL_G = 4  # colacc chunks per tail transpose group
N_GROUPS = -(-(-(-LOC_M // 128)) // TAIL_G)
LOC_PAD = N_GROUPS * TAIL_G * 128  # padded local column space
PAD_X = 64.0  # x-coord of padding points (far away; d2 ~ 4e3)
INIT_F16 = -65504.0  # colacc init (negated-distance identity for max)
CHAIN_G = 8  # tiles per deferred row-min fold group
N_GP = 0  # leading col-min groups on GpSimd (0: SBUF contention hurts)
PW = PW_  # per-tile psum stride, PSUM-bank (512 fp32) safe


def build_program():
    nc = bacc.Bacc()

    lhs_d = nc.dram_tensor("lhs", [K_AUG, NC_N], BF16, kind="ExternalInput").ap()
    rhs_d = nc.dram_tensor("rhs", [K_AUG, LOC_PAD], BF16, kind="ExternalInput").ap()
    ident_d = nc.dram_tensor("ident", [128, 128], FP16, kind="ExternalInput").ap()
    outm_d = nc.dram_tensor(
        "outm", [128, TILES + N_GROUPS * TAIL_G], FP16, kind="ExternalOutput"
    ).ap()
    colgp_d = (
        nc.dram_tensor(
            "colgp", [1, N_GP * TAIL_G * 128], FP16, kind="ExternalOutput"
        ).ap()
        if N_GP
        else None
    )

    amax = mybir.AluOpType.max
    ax_x = mybir.AxisListType.X
    HW = W // 2

    with tile.TileContext(nc) as tc:
        with (
            tc.tile_pool(name="const", bufs=1) as const_pool,
            tc.tile_pool(name="acc", bufs=1) as acc_pool,
            tc.tile_pool(name="drain", bufs=4) as drain_pool,
            tc.tile_pool(name="out", bufs=1) as out_pool,
            tc.tile_pool(name="mm", bufs=3, space="PSUM") as mm_pool,
            tc.tile_pool(name="tr", bufs=2, space="PSUM") as tr_pool,
        ):
            lhs_sb = const_pool.tile([K_AUG, NC_N], BF16)
            rhs_sb = const_pool.tile([K_AUG, LOC_PAD], BF16)
            ident_sb = const_pool.tile([128, 128], FP16)
            # split input DMAs so the first tiles' slices land first; use
            # both hwdge queues (ACT frees earliest) so they overlap
            nc.scalar.dma_start(out=rhs_sb[:, :640], in_=rhs_d[:, :640])
            nc.sync.dma_start(out=lhs_sb[:, :512], in_=lhs_d[:, :512])
            nc.scalar.dma_start(out=rhs_sb[:, 640:1664], in_=rhs_d[:, 640:1664])
            nc.sync.dma_start(out=lhs_sb[:, 512:], in_=lhs_d[:, 512:])
            nc.scalar.dma_start(out=rhs_sb[:, 1664:], in_=rhs_d[:, 1664:])
            nc.sync.dma_start(out=ident_sb, in_=ident_d)

            colacc = acc_pool.tile([128, LOC_PAD], FP16)
            rowhalf = acc_pool.tile([128, TILES * HW], FP16)
            colred = (
                acc_pool.tile([128, N_GP * TAIL_G * 128], FP16) if N_GP else None
            )
            outm_sb = out_pool.tile([128, TILES + N_GROUPS * TAIL_G], FP16)
            rowmin_sb = outm_sb[:, :TILES]
            colmin_sb = outm_sb[:, TILES:]

            # init col accumulator (GpSimd; first window's region first so
            # tile 0 can start folding early)
            nc.gpsimd.memset(colacc[:, :1024], INIT_F16)
            nc.gpsimd.memset(colacc[:, 1024:2048], INIT_F16)
            nc.gpsimd.memset(colacc[:, 2048:], INIT_F16)

            rhv = rowhalf.rearrange("p (t c) -> p t c", c=HW)
            rm = rowmin_sb.rearrange("p (t o) -> p t o", o=1)

            n_tail_emitted = 0
            n_chain_emitted = 0

            def emit_tail(g):
                if g < N_GP:
                    # partition-direction max on the otherwise idle GpSimd
                    s0 = g * TAIL_G * 128
                    s1 = (g + 1) * TAIL_G * 128
                    nc.gpsimd.partition_all_reduce(
                        colred[:, s0:s1],
                        colacc[:, s0:s1],
                        128,
                        bass_isa.ReduceOp.max,
                    )
                    return
                tr_t = tr_pool.tile([128, TAIL_G * 128], FP16, tag="tr")
                for c4 in range(TAIL_G):
                    cc = g * TAIL_G + c4
                    nc.tensor.transpose(
                        tr_t[:, c4 * 128 : (c4 + 1) * 128],
                        colacc[:, cc * 128 : (cc + 1) * 128],
                        ident_sb,
                    )
                nc.vector.tensor_reduce(
                    colmin_sb[:, g * TAIL_G : (g + 1) * TAIL_G],
                    tr_t.rearrange("p (a b) -> p a b", b=128),
                    axis=ax_x,
                    op=amax,
                )

            for q in range(TILES // TPG):  # tile groups
                psum_t = mm_pool.tile([128, TPG * PW], FP32, tag="mm")
                for u in range(TPG):
                    t = TPG * q + u
                    lhs_i = lhs_sb[:, t * 128 : (t + 1) * 128]
                    splits = [512] * (W // 512) + ([W % 512] if W % 512 else [])
                    c0 = t * 128
                    o0 = u * PW  # bank-aligned: matmul must not straddle banks
                    for sz in splits:
                        nc.tensor.matmul(
                            psum_t[:, o0 : o0 + sz],
                            lhs_i,
                            rhs_sb[:, c0 : c0 + sz],
                        )
                        c0 += sz
                        o0 += sz
                drain = drain_pool.tile([128, TPG * PW], FP16)
                # drain with an interleaved layout: [t0a|t1a|t2a|t3a|t0b|..]
                # (a/b = first/second 128 cols of each tile's window) so the
                # row-halving AND both col-folds below are flat 2D ops (2x
                # DVE mode; strided 3D ops fall back to 1x on HW)
                HALF = TPG * HW
                dr_il = drain.rearrange(
                    "p (h u c) -> p u h c", h=2, u=TPG, c=HW
                )
                ps_v = psum_t.rearrange("p (u h c) -> p u h c", h=2, c=HW)
                nc.scalar.mul(dr_il, ps_v, -1.0)  # drain = -d2

                # row-direction halving for all group tiles: flat [128,HALF]
                nc.vector.tensor_tensor(
                    rowhalf[:, TPG * q * HW : TPG * (q + 1) * HW],
                    drain[:, :HALF],
                    drain[:, HALF:],
                    amax,
                )

                # col-direction folds: two flat ops per quad (a-blocks cover
                # cols [q*HALF, q*HALF+HALF), b-blocks shifted by HW)
                base = TPG * q * 128
                sl_a = colacc[:, base : base + HALF]
                nc.vector.tensor_tensor(sl_a, sl_a, drain[:, :HALF], amax)
                sl_b = colacc[:, base + HW : base + HW + HALF]
                nc.vector.tensor_tensor(sl_b, sl_b, drain[:, HALF:], amax)

                # transpose+reduce col groups once final (last touch: tile
                # of last chunk); 4-tile margin for cross-engine slack
                while (
                    n_tail_emitted < N_GP
                    and min((n_tail_emitted + 1) * TAIL_G - 1, TILES - 1)
                    <= TPG * (q + 1) - 1 - 6
                ):
                    emit_tail(n_tail_emitted)
                    n_tail_emitted += 1

                # deferred row-direction fold chains per CHAIN_G tiles
                while (n_chain_emitted + 1) * CHAIN_G <= TPG * (q + 1):
                    j = n_chain_emitted
                    nc.vector.tensor_reduce(
                        rm[:, j * CHAIN_G : (j + 1) * CHAIN_G, :],
                        rhv[:, j * CHAIN_G : (j + 1) * CHAIN_G, :],
                        axis=ax_x,
                        op=amax,
                    )
                    n_chain_emitted += 1

            while n_tail_emitted < N_GROUPS:
                emit_tail(n_tail_emitted)
                n_tail_emitted += 1

            if N_GP:
                nc.sync.dma_start(out=colgp_d, in_=colred[0:1, :])
            nc.sync.dma_start(out=outm_d, in_=outm_sb)

    nc.compile()
    return nc


def _split3(v):
    """v (f64 array) -> (hi, mid, lo) bf16 with hi+mid+lo ~= v (~26-bit)."""
    v = v.astype(np.float64)
    hi = v.astype(NP_BF16)
    r1 = v - hi.astype(np.float64)
    mid = r1.astype(NP_BF16)
    lo = (r1 - mid.astype(np.float64)).astype(NP_BF16)
    return hi, mid, lo


def _make_core_inputs(pts1, pts2):
    """pts1 (NC_N,3), pts2 (LOC_PAD,3) f64 -> lhs [33,NC_N], rhs [33,LOC_PAD] bf16.

    Row pairing (lhs_k paired with rhs_k), ordered so PE partial sums cancel
    early: d2 = sq1 + sq2 - 2*x1.x2 with 3-level splits.
    """
    a1 = _split3(pts1)
    a2 = _split3(pts2)
    n2 = [(-2.0 * p.astype(np.float64)).astype(NP_BF16) for p in a2]  # exact *-2
    sq1 = (pts1 * pts1).sum(-1)
    sq2 = (pts2 * pts2).sum(-1)
    s1 = _split3(sq1)
    s2 = _split3(sq2)

    ones_n = np.ones(pts1.shape[0], NP_BF16)
    ones_m = np.ones(pts2.shape[0], NP_BF16)

    lhs_rows = []
    rhs_rows = []

    def add(l, r):
        lhs_rows.append(l)
        rhs_rows.append(r)

    # big terms first, interleaved for cancellation
    add(s1[0], ones_m)
    for d in range(3):
        add(a1[0][:, d], n2[0][:, d])  # hi*hi
    add(ones_n, s2[0])
    # mid-level terms
    add(s1[1], ones_m)
    add(ones_n, s2[1])
    for d in range(3):
        add(a1[0][:, d], n2[1][:, d])  # hi*mid
    for d in range(3):
        add(a1[1][:, d], n2[0][:, d])  # mid*hi
    for d in range(3):
        add(a1[1][:, d], n2[1][:, d])  # mid*mid
    # low-level terms
    add(s1[2], ones_m)
    add(ones_n, s2[2])
    for d in range(3):
        add(a1[0][:, d], n2[2][:, d])  # hi*lo
    for d in range(3):
        add(a1[2][:, d], n2[0][:, d])  # lo*hi
    for d in range(3):
        add(a1[1][:, d], n2[2][:, d])  # mid*lo
    for d in range(3):
        add(a1[2][:, d], n2[1][:, d])  # lo*mid
    for d in range(3):
        add(a1[2][:, d], n2[2][:, d])  # lo*lo

    lhs = np.ascontiguousarray(np.stack(lhs_rows))
    rhs = np.ascontiguousarray(np.stack(rhs_rows))
    assert lhs.shape == (K_AUG, NC_N) and rhs.shape == (K_AUG, LOC_PAD)
    return lhs, rhs


def _exact_min_d2(a, b):
    """a (k,3), b (n,3) f64 -> (k,) min squared distance via gemm identity."""
    sa = (a * a).sum(-1)[:, None]
    sb = (b * b).sum(-1)[None, :]
    return (sa + sb - 2.0 * (a @ b.T)).min(1)


_CACHED_NC = None


def _get_nc():
    global _CACHED_NC
    if _CACHED_NC is None:
        _CACHED_NC = build_program()
    return _CACHED_NC


def _coverage_rows_for_cols(h, j_global):
    """For sorted col ranks j (array), rows covered by core-half h's windows.

    Returns (r_lo, r_hi) global sorted row ranks [r_lo, r_hi) covered; empty
    coverage gives r_lo >= r_hi.
    """
    loc = j_global + W // 2 - NC_N * h  # local column index
    t_lo = np.maximum((loc - W) // 128 + 1, 0)
    t_hi = np.minimum(loc // 128, TILES - 1)
    valid = (t_lo <= t_hi) & (loc >= 0) & (loc < LOC_M)
    r_lo = np.where(valid, NC_N * h + 128 * t_lo, 0)
    r_hi = np.where(valid, NC_N * h + 128 * t_hi + 128, 0)
    return r_lo, r_hi


def kernel(xyz1, xyz2, _return_timing=False, _trace=False):
    xyz1 = np.asarray(xyz1, dtype=np.float32)
    xyz2 = np.asarray(xyz2, dtype=np.float32)
    assert xyz1.shape == (B, N, 3) and xyz2.shape == (B, M, 3)

    ident = np.eye(128, dtype=np.float16)
    xs1 = []
    xs2 = []
    in_maps = []
    for b in range(B):
        p = xyz1[b].astype(np.float64)
        g = xyz2[b].astype(np.float64)
        o1 = np.argsort(p[:, 0], kind="stable")
        o2 = np.argsort(g[:, 0], kind="stable")
        ps, gs = p[o1], g[o2]
        xs1.append(ps)
        xs2.append(gs)
        for h in range(2):
            rows = ps[h * NC_N : (h + 1) * NC_N]
            # local col l -> global sorted col l - W/2 + NC_N*h; pad outside
            l0 = -(W // 2) + NC_N * h
            cols = np.full((LOC_PAD, 3), 0.0, dtype=np.float64)
            cols[:, 0] = PAD_X
            gidx = np.arange(l0, l0 + LOC_PAD)
            sel = (gidx >= 0) & (gidx < M)
            cols[sel] = gs[gidx[sel]]
            lhs, rhs = _make_core_inputs(rows, cols)
            in_maps.append({"lhs": lhs, "rhs": rhs, "ident": ident})

    nc = _get_nc()
    res = run_bass_kernel_spmd(
        nc, in_maps, core_ids=list(range(N_CORES)), trace=_trace
    )

    total = 0.0
    for b in range(B):
        ps, gs = xs1[b], xs2[b]
        x1, x2 = ps[:, 0], gs[:, 0]

        # ---- row mins (sorted order; device stores -d2) ----
        row_parts = []
        for h in range(2):
            r = res.results[2 * b + h]
            row_parts.append(
                -np.asarray(r["outm"])[:, :TILES].astype(np.float64).T.reshape(-1)
            )
        min1_d2 = np.concatenate(row_parts)  # (8192,) sorted rank order
        min1 = np.sqrt(np.maximum(min1_d2, 0.0))

        # ---- col mins ----
        col_d2 = np.full(M, np.inf)
        for h in range(2):
            r = res.results[2 * b + h]
            pe = (
                -np.asarray(r["outm"])[:, TILES:].astype(np.float64).T.reshape(-1)
            )
            if N_GP:
                gp = -np.asarray(r["colgp"]).astype(np.float64).reshape(-1)
                loc = np.concatenate([gp, pe[N_GP * TAIL_G * 128 :]])
            else:
                loc = pe
            l = np.arange(LOC_PAD)
            gidx = l - W // 2 + NC_N * h
            sel = (l < LOC_M) & (gidx >= 0) & (gidx < M)
            np.minimum.at(col_d2, gidx[sel], loc[sel])
        min2 = np.sqrt(np.maximum(col_d2, 0.0))

        # ---- flag + exact fix: rows ----
        r_rank = np.arange(N)
        t = (r_rank % NC_N) // 128
        h_arr = r_rank // NC_N
        glo = t * 128 + NC_N * h_arr - W // 2
        ghi = glo + W
        c_lo = np.maximum(glo, 0)
        c_hi = np.minimum(ghi, M)
        gapL = np.where(c_lo > 0, x1 - x2[np.maximum(c_lo - 1, 0)], np.inf)
        gapR = np.where(c_hi < M, x2[np.minimum(c_hi, M - 1)] - x1, np.inf)
        gap = np.maximum(np.minimum(gapL, gapR), 0.0)
        idx1 = np.where(min1 > gap * 0.999 - 1e-9)[0]
        if len(idx1):
            min1[idx1] = np.sqrt(np.maximum(_exact_min_d2(ps[idx1], gs), 0.0))

        # ---- flag + exact fix: cols ----
        j = np.arange(M)
        r0_lo, r0_hi = _coverage_rows_for_cols(0, j)
        r1_lo, r1_hi = _coverage_rows_for_cols(1, j)
        # union of [r0_lo,r0_hi) and [r1_lo,r1_hi); empty segments excluded
        e0 = r0_hi > r0_lo
        e1 = r1_hi > r1_lo
        lo_all = np.where(e0, r0_lo, r1_lo)
        hi_all = np.where(e1, r1_hi, r0_hi)
        gapLc = np.where(lo_all > 0, x2 - x1[np.maximum(lo_all - 1, 0)], np.inf)
        gapRc = np.where(hi_all < N, x1[np.minimum(hi_all, N - 1)] - x2, np.inf)
        # middle gap when both segments exist and don't abut
        mid_gap = np.full(M, np.inf)
        mid = e0 & e1 & (r0_hi < r1_lo)
        if mid.any():
            a = np.abs(x1[np.minimum(r0_hi, N - 1)] - x2)
            bb = np.abs(x1[np.maximum(r1_lo - 1, 0)] - x2)
            mid_gap = np.where(mid, np.minimum(a, bb), np.inf)
        gapc = np.maximum(np.minimum(np.minimum(gapLc, gapRc), mid_gap), 0.0)
        idx2 = np.where(min2 > gapc * 0.999 - 1e-9)[0]
        if len(idx2):
            min2[idx2] = np.sqrt(np.maximum(_exact_min_d2(gs[idx2], ps), 0.0))

        total += min1.mean() + min2.mean()

    out = np.asarray(total / B, dtype=np.float32)
    if _return_timing:
        return out, res
    return out
